# revision 21
# baseline (speedup 1.0000x reference)
"""2-layer GCN (PyG GCNConv x2) on 8 Trainium2 NeuronCores via Bass/Tile.

Sharding (per hint): nodes split contiguously across 8 cores (x rows, degree,
output); edges partitioned by destination core so the segment-sum is local;
weight matrices replicated.

v2 structure (vs the f32 baseline):
  - Node positions on each core are split into 4 ranges aligned with the
    gather chunks: table chunk r = AllGather of every core's range r. The
    4 sub-AllGathers (Shared outputs, the NRT fast path) fire as the
    transform finishes each range, so chunk-0 aggregation overlaps the
    remaining collectives.
  - Tables are bf16 [rows, 128]: row stride stays 256 B (dma_gather
    requirement) with pad channels that are never read. Messages, one-hot
    masks, and the input transform all run in bf16 (2x DVE and PE), with
    PSUM accumulation in f32.
  - Pad gather positions use slot=-1: their one-hot column is all zero, so
    they contribute nothing regardless of the row they fetch (no dedicated
    zero row needed).

Per core: transform-first hT = W1^T @ xT on PE, scaled by deg^-1/2, written
node-major to a DRAM shard; 4 sub-AllGathers build the 64-ch (+pad) table;
local segment-sum via int16-indexed dma_gather ([128, Q, 128] bf16 tiles) and
one-hot matmul accumulation in PSUM; epilogue x dinv[dst] + bias (+relu);
layer 2 repeats with W2/b2 after a second set of sub-AllGathers.
"""

import numpy as np

P = 128
N_CORES = 8
HID = 64
TW = 128         # table row width (bf16) -> 256 B stride for dma_gather
QG = 122         # max 128-edge groups per gather call
NQ = 2           # SWDGE queues; gather calls round-robin across them
R_SIZES = [3200, 3200, 3072, 3072]   # per-core range sizes (25+25+24+24 blocks)


# ----------------------------------------------------------------- host prep
def _preprocess(x, edge_index, n_cores=N_CORES, qg=QG):
    import ml_dtypes
    bf16 = ml_dtypes.bfloat16

    n_nodes = x.shape[0]
    npc = n_nodes // n_cores
    assert npc * n_cores == n_nodes
    nb = -(-npc // P)
    shard = nb * P
    rs = np.asarray(R_SIZES, dtype=np.int64)
    assert rs.sum() == shard and all(s % P == 0 for s in rs)
    n_chunks = len(rs)
    range_start = np.concatenate([[0], np.cumsum(rs)[:-1]])
    chunk_rows = [int(8 * s) for s in rs]
    chunk_bases = np.concatenate([[0], np.cumsum(chunk_rows)[:-1]]).astype(np.int64)
    total = n_cores * shard
    assert max(chunk_rows) <= 32768  # int16 gather index range

    src = np.asarray(edge_index[0], dtype=np.int64)
    dst = np.asarray(edge_index[1], dtype=np.int64)
    deg = np.bincount(dst, minlength=n_nodes).astype(np.int64) + 1
    dinv = (1.0 / np.sqrt(deg.astype(np.float64))).astype(np.float32)

    loop = np.arange(n_nodes, dtype=np.int64)
    src_all = np.concatenate([src, loop])
    dst_all = np.concatenate([dst, loop])

    sorted_nodes = []
    row_of = np.empty(n_nodes, dtype=np.int64)
    pos_of = np.empty(n_nodes, dtype=np.int64)
    bounds = np.cumsum(rs)[:-1]
    for k in range(n_cores):
        nodes_k = np.arange(k * npc, (k + 1) * npc)
        order = np.argsort(-deg[nodes_k], kind="stable")
        sn = nodes_k[order]
        sorted_nodes.append(sn)
        p = np.arange(npc)
        r = np.searchsorted(bounds, p, side="right")
        row_of[sn] = chunk_bases[r] + k * rs[r] + (p - range_start[r])
        pos_of[sn] = p

    # per-core edges keyed by (chunk, dst position); per-(b, c) counts
    e_owner = dst_all // npc
    per_core = []
    cnt_bc = np.zeros((n_cores, nb, n_chunks), dtype=np.int64)
    for k in range(n_cores):
        m = e_owner == k
        p_e = pos_of[dst_all[m]]
        sr_e = row_of[src_all[m]]
        c_e = np.searchsorted(chunk_bases[1:], sr_e, side="right")
        key = c_e * shard + p_e            # chunk-major, then dst position
        perm = np.argsort(key, kind="stable")
        p_s, c_s, sr_s = p_e[perm], c_e[perm], sr_e[perm]
        b_s = p_s // P
        cb = np.bincount(c_s * nb + b_s, minlength=n_chunks * nb)
        cnt_bc[k] = cb.reshape(n_chunks, nb).T
        per_core.append((p_s, c_s, sr_s))

    # global group counts per (block, chunk)
    G = np.zeros((nb, n_chunks), dtype=np.int64)
    for b in range(nb):
        for c in range(n_chunks):
            mx = int(cnt_bc[:, b, c].max())
            G[b, c] = -(-mx // P) if mx else 0
    n_pos = int(P * G.sum())

    first_c, last_c = {}, {}
    for b in range(nb):
        nz = np.nonzero(G[b])[0]
        assert len(nz) > 0
        first_c[b] = int(nz[0])
        last_c[b] = int(nz[-1])

    # call plan: chunk-major, pack (b, c) runs into calls of <= qg groups
    calls = []
    for c in range(n_chunks):
        cur, lo = [], 0
        for b in range(nb):
            g = int(G[b, c])
            if g == 0:
                continue
            assert g <= qg, f"G[{b},{c}]={g} exceeds qg"
            if lo + g > qg:
                calls.append((c, cur))
                cur, lo = [], 0
            cur.append((b, lo, g))
            lo += g
        if cur:
            calls.append((c, cur))

    # run base offsets (in edges) inside the global padded stream
    run_base = np.zeros((nb, n_chunks), dtype=np.int64)
    pos = 0
    for c, blocks in calls:
        for b, lo, g in blocks:
            run_base[b, c] = pos
            pos += g * P
    assert pos == n_pos

    gidx, slots = [], []
    for k in range(n_cores):
        p_s, c_s, sr_s = per_core[k]
        b_s = p_s // P
        # rank of each edge within its (b, c) run
        key2 = c_s * nb + b_s
        cb = np.bincount(key2, minlength=n_chunks * nb)
        starts = np.zeros(n_chunks * nb, dtype=np.int64)
        starts[1:] = np.cumsum(cb)[:-1]
        j = np.arange(len(p_s)) - starts[key2]
        flat_pos = run_base[b_s, c_s] + j
        idx_flat = np.zeros(n_pos, dtype=np.int16)       # pads fetch row 0
        slot_flat = np.full(n_pos, -1.0, dtype=np.float32)  # pads match no slot
        idx_flat[flat_pos] = (sr_s - chunk_bases[c_s]).astype(np.int16)
        slot_flat[flat_pos] = (p_s % P).astype(np.float32)
        # wrapped int16 stream: per call [128, 8*Q]
        segs = []
        for c, blocks in calls:
            qn = sum(g for _, _, g in blocks)
            rb = run_base[blocks[0][0], c]
            fl = idx_flat[rb:rb + qn * P]
            segs.append(np.tile(fl.reshape(-1, 16).T, (8, 1)))
        gidx.append(np.ascontiguousarray(np.concatenate(segs, axis=1)))
        # slot columns [128, n_groups] bf16
        slots.append(np.ascontiguousarray(
            slot_flat.reshape(-1, P).T.astype(bf16)))

    dinv_cols, xts = [], []
    for k in range(n_cores):
        tmp = np.zeros(shard, dtype=np.float32)
        tmp[:npc] = dinv[sorted_nodes[k]]
        dinv_cols.append(np.ascontiguousarray(tmp.reshape(nb, P).T))
        xt = np.zeros((x.shape[1], shard), dtype=bf16)
        xt[:, :npc] = np.asarray(x, dtype=np.float32)[sorted_nodes[k]].T
        xts.append(xt)

    iota = np.tile(np.arange(P, dtype=np.float32)[None, :], (P, 1)).astype(bf16)

    return dict(
        n_nodes=n_nodes, npc=npc, nb=nb, shard=shard, n_chunks=n_chunks,
        chunk_bases=[int(v) for v in chunk_bases], chunk_rows=chunk_rows,
        calls=calls, first_c=first_c, last_c=last_c, wtot=gidx[0].shape[1],
        n_groups=int(G.sum()), n_pos=n_pos, iota=iota,
        gidx=gidx, slots=slots, dinv_cols=dinv_cols, xts=xts,
        sorted_nodes=sorted_nodes,
    )


# ------------------------------------------------------------- bass program
def _build(in_ch, meta, n_cores=N_CORES, stage=4, nq=NQ, ts_oh=False,
           gat_bufs=3):
    import concourse.bacc as bacc
    import concourse.tile as tile
    from concourse import mybir
    from concourse.masks import make_identity

    f32 = mybir.dt.float32
    bf16 = mybir.dt.bfloat16
    i16 = mybir.dt.int16
    kc = in_ch // P
    rg = [list(range(n_cores))]
    shard, nb = meta["shard"], meta["nb"]
    calls, first_c, last_c = meta["calls"], meta["first_c"], meta["last_c"]
    chunk_bases, chunk_rows = meta["chunk_bases"], meta["chunk_rows"]
    wtot, n_groups = meta["wtot"], meta["n_groups"]
    n_chunks = meta["n_chunks"]
    total = n_cores * shard
    rs = R_SIZES
    rstart = [0]
    for s in rs[:-1]:
        rstart.append(rstart[-1] + s)
    # block index (exclusive) after which range r is fully written
    rend_blk = []
    acc_ = 0
    for s in rs:
        acc_ += s
        rend_blk.append(acc_ // P)

    nc = bacc.Bacc("TRN2", target_bir_lowering=False, debug=False,
                   num_devices=n_cores, num_swdge_queues=nq)
    xT_d = nc.dram_tensor("xT", [in_ch, shard], bf16, kind="ExternalInput").ap()
    gidx_d = nc.dram_tensor("gidx", [P, wtot], i16, kind="ExternalInput").ap()
    slot_d = nc.dram_tensor("slots", [P, n_groups], bf16,
                            kind="ExternalInput").ap()
    iota_d = nc.dram_tensor("iota", [P, P], bf16, kind="ExternalInput").ap()
    dinv_d = nc.dram_tensor("dinv", [P, nb], f32, kind="ExternalInput").ap()
    w1_d = nc.dram_tensor("W1", [in_ch, HID], bf16, kind="ExternalInput").ap()
    b1_d = nc.dram_tensor("b1", [1, HID], f32, kind="ExternalInput").ap()
    w2_d = nc.dram_tensor("W2", [HID, HID], f32, kind="ExternalInput").ap()
    b2_d = nc.dram_tensor("b2", [1, HID], f32, kind="ExternalInput").ap()
    out_d = nc.dram_tensor("out", [shard, HID], f32, kind="ExternalOutput").ap()

    table1 = nc.dram_tensor("table1", [total, TW], bf16, kind="Internal",
                            addr_space="Shared").ap()
    table2 = nc.dram_tensor("table2", [total, TW], bf16, kind="Internal",
                            addr_space="Shared").ap()

    with tile.TileContext(nc) as tc:
        with tc.tile_pool(name="const", bufs=1) as cp, \
             tc.tile_pool(name="sb", bufs=3) as sb, \
             tc.tile_pool(name="red", bufs=4) as rp, \
             tc.tile_pool(name="oh", bufs=3) as ohp, \
             tc.tile_pool(name="gat", bufs=gat_bufs) as gp, \
             tc.tile_pool(name="accp", bufs=1) as ap_, \
             tc.tile_pool(name="ps", bufs=8, space="PSUM") as pp, \
             tc.tile_pool(name="dram", bufs=1, space="DRAM") as dp:

            w1 = cp.tile([P, kc, HID], bf16)
            nc.sync.dma_start(w1[:], w1_d.rearrange("(c p) h -> p c h", p=P))
            w2 = cp.tile([HID, HID], f32)
            nc.sync.dma_start(w2[:], w2_d[:])
            ident = cp.tile([P, P], f32)
            make_identity(nc, ident[:])
            iota_sb = cp.tile([P, P], bf16)
            nc.sync.dma_start(iota_sb[:], iota_d[:])
            dinv_sb = cp.tile([P, nb], f32)
            nc.sync.dma_start(dinv_sb[:], dinv_d[:])
            b1_row = cp.tile([1, HID], f32)
            nc.sync.dma_start(b1_row[:], b1_d[:])
            b1_bc = cp.tile([P, HID], f32)
            nc.gpsimd.partition_broadcast(b1_bc[:], b1_row[:])
            b2_row = cp.tile([1, HID], f32)
            nc.sync.dma_start(b2_row[:], b2_d[:])
            b2_bc = cp.tile([P, HID], f32)
            nc.gpsimd.partition_broadcast(b2_bc[:], b2_row[:])

            shard1 = dp.tile([shard, TW], bf16)
            shard2 = dp.tile([shard, TW], bf16)

            # whole index/slot streams resident in SBUF for both layers
            gidx_all = cp.tile([P, wtot], i16)
            nc.sync.dma_start(gidx_all[:], gidx_d[:])
            slots_all = cp.tile([P, n_groups], bf16)
            nc.sync.dma_start(slots_all[:], slot_d[:])
            if ts_oh:
                slots_f = cp.tile([P, n_groups], f32)
                nc.scalar.copy(slots_f[:], slots_all[:])

            def sub_ag(shard_t, table_t, r, tag):
                nc.gpsimd.collective_compute(
                    "AllGather", mybir.AluOpType.bypass, replica_groups=rg,
                    ins=[shard_t[rstart[r]:rstart[r] + rs[r], :].opt()],
                    outs=[table_t[chunk_bases[r]:
                                  chunk_bases[r] + chunk_rows[r], :].opt()])

            # ---- layer-1 transform: h1' = dinv * (x @ W1), node-major, bf16.
            xT_r = xT_d.rearrange("(c p) n -> p c n", p=P)
            pairs = [(t, min(t + 2, nb)) for t in range(0, nb, 2)]
            ag_done = 0
            for t0, t1 in pairs:
                w = (t1 - t0) * P
                xt = sb.tile([P, kc, 2 * P], bf16, tag="xt")
                nc.sync.dma_start(xt[:, :, :w],
                                  xT_r[:, :, t0 * P:t0 * P + w])
                hT = pp.tile([HID, 2 * P], f32, tag="ps")
                for c in range(kc):
                    nc.tensor.matmul(out=hT[:, :w], lhsT=w1[:, c, :],
                                     rhs=xt[:, c, :w],
                                     start=(c == 0), stop=(c == kc - 1))
                hTs = sb.tile([HID, 2 * P], f32, tag="hTs")
                nc.scalar.copy(hTs[:, :w], hT[:, :w])
                for t in range(t0, t1):
                    off = (t - t0) * P
                    h = pp.tile([P, HID], f32, tag="ps")
                    nc.tensor.transpose(out=h[:], in_=hTs[:, off:off + P],
                                        identity=ident[:HID, :HID])
                    hp = sb.tile([P, HID], bf16, tag="hp")
                    nc.scalar.mul(hp[:], h[:], mul=dinv_sb[:, t:t + 1])
                    nc.sync.dma_start(shard1[t * P:(t + 1) * P, :HID], hp[:])
                while ag_done < n_chunks and t1 >= rend_blk[ag_done]:
                    if stage != 0:
                        sub_ag(shard1, table1, ag_done, "ag1")
                    ag_done += 1

            if stage == 0:
                for b in range(nb):
                    t0_ = sb.tile([P, HID], bf16, tag="cp")
                    nc.sync.dma_start(t0_[:], shard1[b * P:(b + 1) * P, :HID])
                    o_ = sb.tile([P, HID], f32, tag="cpo")
                    nc.scalar.copy(o_[:], t0_[:])
                    nc.sync.dma_start(out_d[b * P:(b + 1) * P, :], o_[:])

            def aggregate(table, layer, gather_only=False, finish=None,
                          after_block=None):
                colpos = 0
                gpos = 0
                qsel = 0
                acc = ap_.tile([P, nb * HID], f32, tag=f"acc{layer}",
                               name=f"acc{layer}")
                done = []
                for c, blocks in calls:
                    qn = sum(g for _, _, g in blocks)
                    gidx_sb = gidx_all[:, colpos:colpos + 8 * qn]
                    colpos += 8 * qn
                    slots_sb = slots_all[:, gpos:gpos + qn]
                    gt = gp.tile([P, qn, TW], bf16, tag="g")
                    nc.gpsimd.dma_gather(
                        out_ap=gt[:],
                        in_ap=table[chunk_bases[c]:
                                    chunk_bases[c] + chunk_rows[c], :],
                        idxs_ap=gidx_sb, num_idxs=P * qn,
                        num_idxs_reg=P * qn, elem_size=TW,
                        single_packet=False, queue_num=qsel)
                    qsel = (qsel + 1) % nq
                    if gather_only:
                        gpos += qn
                        continue
                    for b, lo, g in blocks:
                        ps = pp.tile([P, HID], f32, tag="ps")
                        oh = ohp.tile([P, g, P], bf16, tag="oh")
                        if ts_oh:
                            for q in range(g):
                                nc.vector.tensor_scalar(
                                    out=oh[:, q, :], in0=iota_sb[:],
                                    scalar1=slots_f[:, gpos + lo + q:
                                                    gpos + lo + q + 1],
                                    scalar2=None,
                                    op0=mybir.AluOpType.is_equal)
                        else:
                            nc.vector.tensor_tensor(
                                out=oh[:],
                                in0=iota_sb[:].rearrange("p (g j) -> p g j", g=1)
                                    .to_broadcast([P, g, P]),
                                in1=slots_sb[:, lo:lo + g]
                                    .rearrange("p (g j) -> p g j", j=1)
                                    .to_broadcast([P, g, P]),
                                op=mybir.AluOpType.is_equal)
                        for q in range(lo, lo + g):
                            nc.tensor.matmul(out=ps[:], lhsT=oh[:, q - lo, :],
                                             rhs=gt[:, q, :HID],
                                             start=(q == lo),
                                             stop=(q == lo + g - 1))
                        a_sl = acc[:, b * HID:(b + 1) * HID]
                        if c == first_c[b]:
                            nc.scalar.copy(a_sl, ps[:])
                        else:
                            nc.vector.tensor_add(out=a_sl, in0=a_sl,
                                                 in1=ps[:])
                        if c == last_c[b]:
                            if finish is not None:
                                finish(b, a_sl)
                            if after_block is not None:
                                after_block(b)
                            if finish is None:
                                done.append((b, a_sl))
                    gpos += qn
                return done

            # ---- layer-1 aggregation + layer-2 transform (inline finish)
            def finish1(b, a_sl):
                accb = rp.tile([P, HID], f32, tag="accb", name="accb")
                nc.vector.tensor_scalar(
                    out=accb[:], in0=a_sl, scalar1=dinv_sb[:, b:b + 1],
                    scalar2=None, op0=mybir.AluOpType.mult)
                acc2 = rp.tile([P, HID], f32, tag="acc2", name="acc2")
                nc.vector.tensor_add(out=acc2[:], in0=accb[:], in1=b1_bc[:])
                h2 = sb.tile([P, HID], f32, tag="h2", bufs=6, name="h2")
                nc.scalar.activation(h2[:], acc2[:],
                                     mybir.ActivationFunctionType.Relu)
                h2T = pp.tile([HID, P], f32, tag="ps", name="h2T")
                nc.tensor.transpose(out=h2T[:], in_=h2[:], identity=ident[:])
                h2Ts = sb.tile([HID, P], f32, tag="h2Ts", bufs=6, name="h2Ts")
                nc.scalar.copy(h2Ts[:], h2T[:])
                gT = pp.tile([HID, P], f32, tag="ps", name="gT")
                nc.tensor.matmul(out=gT[:], lhsT=w2[:], rhs=h2Ts[:],
                                 start=True, stop=True)
                gTs = sb.tile([HID, P], f32, tag="gTs", bufs=6, name="gTs")
                nc.scalar.copy(gTs[:], gT[:])
                gg = pp.tile([P, HID], f32, tag="ps", name="gg")
                nc.tensor.transpose(out=gg[:], in_=gTs[:],
                                    identity=ident[:HID, :HID])
                gsb = sb.tile([P, HID], bf16, tag="gsb", bufs=6, name="gsb")
                nc.scalar.mul(gsb[:], gg[:], mul=dinv_sb[:, b:b + 1])
                nc.sync.dma_start(shard2[b * P:(b + 1) * P, :HID], gsb[:])

            def finish3(b, a_sl):
                ob = rp.tile([P, HID], f32, tag="ob", name="ob")
                nc.vector.tensor_scalar(
                    out=ob[:], in0=a_sl, scalar1=dinv_sb[:, b:b + 1],
                    scalar2=None, op0=mybir.AluOpType.mult)
                nc.sync.dma_start(out_d[b * P:(b + 1) * P, :], ob[:])

            ag2_state = {"done": 0}

            def after_block1(b):
                while (ag2_state["done"] < n_chunks
                       and b + 1 >= rend_blk[ag2_state["done"]]):
                    sub_ag(shard2, table2, ag2_state["done"], "ag2")
                    ag2_state["done"] += 1

            if stage == 2:
                aggregate(table1, 1, gather_only=True)
            elif stage == 3:
                aggregate(table1, 1, finish=finish3)
            elif stage >= 4:
                aggregate(table1, 1, finish=finish1, after_block=after_block1)

            # ---- layer-2 aggregation (inline final epilogue)
            def finish2(b, a_sl):
                acc4 = rp.tile([P, HID], f32, tag="acc4", name="acc4")
                nc.vector.tensor_scalar(
                    out=acc4[:], in0=a_sl, scalar1=dinv_sb[:, b:b + 1],
                    scalar2=None, op0=mybir.AluOpType.mult)
                osb = sb.tile([P, HID], f32, tag="osb", name="osb")
                nc.vector.tensor_add(out=osb[:], in0=acc4[:], in1=b2_bc[:])
                nc.sync.dma_start(out_d[b * P:(b + 1) * P, :], osb[:])

            if stage >= 4:
                aggregate(table2, 2, finish=finish2)

    nc.compile()
    return nc


# ------------------------------------------------------------------- driver
_CACHE = {}


def _get_nc(in_ch, meta):
    key = (in_ch, meta["shard"], meta["wtot"],
           tuple((c, tuple(b)) for c, bl in meta["calls"] for b in bl))
    if key not in _CACHE:
        _CACHE[key] = _build(in_ch, meta)
    return _CACHE[key]


def _in_maps(pre, W1, b1, W2, b2):
    import ml_dtypes
    maps = []
    for k in range(N_CORES):
        maps.append({
            "xT": pre["xts"][k],
            "gidx": pre["gidx"][k],
            "slots": pre["slots"][k],
            "iota": pre["iota"],
            "dinv": pre["dinv_cols"][k],
            "W1": np.ascontiguousarray(W1.astype(ml_dtypes.bfloat16)),
            "b1": b1, "W2": W2, "b2": b2,
        })
    return maps


def kernel(x, edge_index, W1, b1, W2, b2):
    from concourse.bass_utils import run_bass_kernel_spmd

    x = np.asarray(x, dtype=np.float32)
    W1 = np.ascontiguousarray(np.asarray(W1, dtype=np.float32))
    W2 = np.ascontiguousarray(np.asarray(W2, dtype=np.float32))
    b1 = np.asarray(b1, dtype=np.float32).reshape(1, HID)
    b2 = np.asarray(b2, dtype=np.float32).reshape(1, HID)

    pre = _preprocess(x, edge_index)
    nc = _get_nc(x.shape[1], pre)
    res = run_bass_kernel_spmd(nc, _in_maps(pre, W1, b1, W2, b2),
                               core_ids=list(range(N_CORES)))

    npc = pre["npc"]
    out = np.empty((pre["n_nodes"], HID), dtype=np.float32)
    for k in range(N_CORES):
        out[pre["sorted_nodes"][k]] = res.results[k]["out"][:npc]
    return out


# revision 22
# speedup vs baseline: 1.1577x; 1.1577x over previous
"""2-layer GCN (PyG GCNConv x2) on 8 Trainium2 NeuronCores via Bass/Tile.

Sharding (per hint): nodes split contiguously across 8 cores (x rows, degree,
output); edges partitioned by destination core so the segment-sum is local;
weight matrices replicated.

v2 structure (vs the f32 baseline):
  - Node positions on each core are split into 4 ranges aligned with the
    gather chunks: table chunk r = AllGather of every core's range r. The
    4 sub-AllGathers (Shared outputs, the NRT fast path) fire as the
    transform finishes each range, so chunk-0 aggregation overlaps the
    remaining collectives.
  - Tables are bf16 [rows, 128]: row stride stays 256 B (dma_gather
    requirement) with pad channels that are never read. Messages, one-hot
    masks, and the input transform all run in bf16 (2x DVE and PE), with
    PSUM accumulation in f32.
  - Pad gather positions use slot=-1: their one-hot column is all zero, so
    they contribute nothing regardless of the row they fetch (no dedicated
    zero row needed).

Per core: transform-first hT = W1^T @ xT on PE, scaled by deg^-1/2, written
node-major to a DRAM shard; 4 sub-AllGathers build the 64-ch (+pad) table;
local segment-sum via int16-indexed dma_gather ([128, Q, 128] bf16 tiles) and
one-hot matmul accumulation in PSUM; epilogue x dinv[dst] + bias (+relu);
layer 2 repeats with W2/b2 after a second set of sub-AllGathers.
"""

import numpy as np

P = 128
N_CORES = 8
HID = 64
TW = 128         # table row width (bf16) -> 256 B stride for dma_gather
QG = 110         # max 128-edge groups per gather call
NQ = 2           # SWDGE queues; gather calls round-robin across them
R_SIZES = [3200, 3200, 3072, 3072]   # per-core range sizes (25+25+24+24 blocks)


# ----------------------------------------------------------------- host prep
def _preprocess(x, edge_index, n_cores=N_CORES, qg=QG):
    import ml_dtypes
    bf16 = ml_dtypes.bfloat16

    n_nodes = x.shape[0]
    npc = n_nodes // n_cores
    assert npc * n_cores == n_nodes
    nb = -(-npc // P)
    shard = nb * P
    rs = np.asarray(R_SIZES, dtype=np.int64)
    assert rs.sum() == shard and all(s % P == 0 for s in rs)
    n_chunks = len(rs)
    range_start = np.concatenate([[0], np.cumsum(rs)[:-1]])
    chunk_rows = [int(8 * s) for s in rs]
    chunk_bases = np.concatenate([[0], np.cumsum(chunk_rows)[:-1]]).astype(np.int64)
    total = n_cores * shard
    assert max(chunk_rows) <= 32768  # int16 gather index range

    src = np.asarray(edge_index[0], dtype=np.int64)
    dst = np.asarray(edge_index[1], dtype=np.int64)
    deg = np.bincount(dst, minlength=n_nodes).astype(np.int64) + 1
    dinv = (1.0 / np.sqrt(deg.astype(np.float64))).astype(np.float32)

    loop = np.arange(n_nodes, dtype=np.int64)
    src_all = np.concatenate([src, loop])
    dst_all = np.concatenate([dst, loop])

    sorted_nodes = []
    row_of = np.empty(n_nodes, dtype=np.int64)
    pos_of = np.empty(n_nodes, dtype=np.int64)
    bounds = np.cumsum(rs)[:-1]
    for k in range(n_cores):
        nodes_k = np.arange(k * npc, (k + 1) * npc)
        order = np.argsort(-deg[nodes_k], kind="stable")
        sn = nodes_k[order]
        sorted_nodes.append(sn)
        p = np.arange(npc)
        r = np.searchsorted(bounds, p, side="right")
        row_of[sn] = chunk_bases[r] + k * rs[r] + (p - range_start[r])
        pos_of[sn] = p

    # per-core edges keyed by (chunk, dst position); per-(b, c) counts
    e_owner = dst_all // npc
    per_core = []
    cnt_bc = np.zeros((n_cores, nb, n_chunks), dtype=np.int64)
    for k in range(n_cores):
        m = e_owner == k
        p_e = pos_of[dst_all[m]]
        sr_e = row_of[src_all[m]]
        c_e = np.searchsorted(chunk_bases[1:], sr_e, side="right")
        key = c_e * shard + p_e            # chunk-major, then dst position
        perm = np.argsort(key, kind="stable")
        p_s, c_s, sr_s = p_e[perm], c_e[perm], sr_e[perm]
        b_s = p_s // P
        cb = np.bincount(c_s * nb + b_s, minlength=n_chunks * nb)
        cnt_bc[k] = cb.reshape(n_chunks, nb).T
        per_core.append((p_s, c_s, sr_s))

    # global group counts per (block, chunk)
    G = np.zeros((nb, n_chunks), dtype=np.int64)
    for b in range(nb):
        for c in range(n_chunks):
            mx = int(cnt_bc[:, b, c].max())
            G[b, c] = -(-mx // P) if mx else 0
    n_pos = int(P * G.sum())

    first_c, last_c = {}, {}
    for b in range(nb):
        nz = np.nonzero(G[b])[0]
        assert len(nz) > 0
        first_c[b] = int(nz[0])
        last_c[b] = int(nz[-1])

    # call plan: chunk-major, pack (b, c) runs into calls of <= qg groups
    calls = []
    for c in range(n_chunks):
        cur, lo = [], 0
        for b in range(nb):
            g = int(G[b, c])
            if g == 0:
                continue
            assert g <= qg, f"G[{b},{c}]={g} exceeds qg"
            if lo + g > qg:
                calls.append((c, cur))
                cur, lo = [], 0
            cur.append((b, lo, g))
            lo += g
        if cur:
            calls.append((c, cur))

    # run base offsets (in edges) inside the global padded stream
    run_base = np.zeros((nb, n_chunks), dtype=np.int64)
    pos = 0
    for c, blocks in calls:
        for b, lo, g in blocks:
            run_base[b, c] = pos
            pos += g * P
    assert pos == n_pos

    gidx, slots = [], []
    for k in range(n_cores):
        p_s, c_s, sr_s = per_core[k]
        b_s = p_s // P
        # rank of each edge within its (b, c) run
        key2 = c_s * nb + b_s
        cb = np.bincount(key2, minlength=n_chunks * nb)
        starts = np.zeros(n_chunks * nb, dtype=np.int64)
        starts[1:] = np.cumsum(cb)[:-1]
        j = np.arange(len(p_s)) - starts[key2]
        flat_pos = run_base[b_s, c_s] + j
        idx_flat = np.zeros(n_pos, dtype=np.int16)       # pads fetch row 0
        slot_flat = np.full(n_pos, -1.0, dtype=np.float32)  # pads match no slot
        idx_flat[flat_pos] = (sr_s - chunk_bases[c_s]).astype(np.int16)
        slot_flat[flat_pos] = (p_s % P).astype(np.float32)
        # wrapped int16 stream: per call [128, 8*Q]
        segs = []
        for c, blocks in calls:
            qn = sum(g for _, _, g in blocks)
            rb = run_base[blocks[0][0], c]
            fl = idx_flat[rb:rb + qn * P]
            segs.append(np.tile(fl.reshape(-1, 16).T, (8, 1)))
        gidx.append(np.ascontiguousarray(np.concatenate(segs, axis=1)))
        # slot columns [128, n_groups] bf16
        slots.append(np.ascontiguousarray(
            slot_flat.reshape(-1, P).T.astype(bf16)))

    dinv_cols, xts = [], []
    for k in range(n_cores):
        tmp = np.zeros(shard, dtype=np.float32)
        tmp[:npc] = dinv[sorted_nodes[k]]
        dinv_cols.append(np.ascontiguousarray(tmp.reshape(nb, P).T))
        xt = np.zeros((x.shape[1], shard), dtype=bf16)
        xt[:, :npc] = np.asarray(x, dtype=np.float32)[sorted_nodes[k]].T
        xts.append(xt)

    iota = np.tile(np.arange(P, dtype=np.float32)[None, :], (P, 1)).astype(bf16)

    return dict(
        n_nodes=n_nodes, npc=npc, nb=nb, shard=shard, n_chunks=n_chunks,
        chunk_bases=[int(v) for v in chunk_bases], chunk_rows=chunk_rows,
        calls=calls, first_c=first_c, last_c=last_c, wtot=gidx[0].shape[1],
        n_groups=int(G.sum()), n_pos=n_pos, iota=iota,
        gidx=gidx, slots=slots, dinv_cols=dinv_cols, xts=xts,
        sorted_nodes=sorted_nodes,
    )


# ------------------------------------------------------------- bass program
def _build(in_ch, meta, n_cores=N_CORES, stage=4, nq=NQ, ts_oh=False,
           gat_bufs=3):
    import concourse.bacc as bacc
    import concourse.tile as tile
    from concourse import mybir
    from concourse.masks import make_identity

    f32 = mybir.dt.float32
    bf16 = mybir.dt.bfloat16
    i16 = mybir.dt.int16
    kc = in_ch // P
    rg = [list(range(n_cores))]
    shard, nb = meta["shard"], meta["nb"]
    calls, first_c, last_c = meta["calls"], meta["first_c"], meta["last_c"]
    chunk_bases, chunk_rows = meta["chunk_bases"], meta["chunk_rows"]
    wtot, n_groups = meta["wtot"], meta["n_groups"]
    n_chunks = meta["n_chunks"]
    total = n_cores * shard
    rs = R_SIZES
    rstart = [0]
    for s in rs[:-1]:
        rstart.append(rstart[-1] + s)
    # block index (exclusive) after which range r is fully written
    rend_blk = []
    acc_ = 0
    for s in rs:
        acc_ += s
        rend_blk.append(acc_ // P)

    nc = bacc.Bacc("TRN2", target_bir_lowering=False, debug=False,
                   num_devices=n_cores, num_swdge_queues=nq)
    xT_d = nc.dram_tensor("xT", [in_ch, shard], bf16, kind="ExternalInput").ap()
    gidx_d = nc.dram_tensor("gidx", [P, wtot], i16, kind="ExternalInput").ap()
    slot_d = nc.dram_tensor("slots", [P, n_groups], bf16,
                            kind="ExternalInput").ap()
    iota_d = nc.dram_tensor("iota", [P, P], bf16, kind="ExternalInput").ap()
    dinv_d = nc.dram_tensor("dinv", [P, nb], f32, kind="ExternalInput").ap()
    w1_d = nc.dram_tensor("W1", [in_ch, HID], bf16, kind="ExternalInput").ap()
    b1_d = nc.dram_tensor("b1", [1, HID], f32, kind="ExternalInput").ap()
    w2_d = nc.dram_tensor("W2", [HID, HID], f32, kind="ExternalInput").ap()
    b2_d = nc.dram_tensor("b2", [1, HID], f32, kind="ExternalInput").ap()
    out_d = nc.dram_tensor("out", [shard, HID], f32, kind="ExternalOutput").ap()

    table1 = nc.dram_tensor("table1", [total, TW], bf16, kind="Internal",
                            addr_space="Shared").ap()
    table2 = nc.dram_tensor("table2", [total, TW], bf16, kind="Internal",
                            addr_space="Shared").ap()

    with tile.TileContext(nc) as tc:
        with tc.tile_pool(name="const", bufs=1) as cp, \
             tc.tile_pool(name="sb", bufs=3) as sb, \
             tc.tile_pool(name="red", bufs=4) as rp, \
             tc.tile_pool(name="oh", bufs=4) as ohp, \
             tc.tile_pool(name="gat", bufs=gat_bufs) as gp, \
             tc.tile_pool(name="accp", bufs=1) as ap_, \
             tc.tile_pool(name="ps", bufs=8, space="PSUM") as pp, \
             tc.tile_pool(name="dram", bufs=1, space="DRAM") as dp:

            w1 = cp.tile([P, kc, HID], bf16)
            nc.sync.dma_start(w1[:], w1_d.rearrange("(c p) h -> p c h", p=P))
            w2 = cp.tile([HID, HID], f32)
            nc.sync.dma_start(w2[:], w2_d[:])
            ident = cp.tile([P, P], f32)
            make_identity(nc, ident[:])
            iota_sb = cp.tile([P, P], bf16)
            nc.sync.dma_start(iota_sb[:], iota_d[:])
            dinv_sb = cp.tile([P, nb], f32)
            nc.sync.dma_start(dinv_sb[:], dinv_d[:])
            b1_row = cp.tile([1, HID], f32)
            nc.sync.dma_start(b1_row[:], b1_d[:])
            b1_bc = cp.tile([P, HID], f32)
            nc.gpsimd.partition_broadcast(b1_bc[:], b1_row[:])
            b2_row = cp.tile([1, HID], f32)
            nc.sync.dma_start(b2_row[:], b2_d[:])
            b2_bc = cp.tile([P, HID], f32)
            nc.gpsimd.partition_broadcast(b2_bc[:], b2_row[:])

            shard1 = dp.tile([shard, TW], bf16)
            shard2 = dp.tile([shard, TW], bf16)

            # whole index/slot streams resident in SBUF for both layers
            gidx_all = cp.tile([P, wtot], i16)
            nc.sync.dma_start(gidx_all[:], gidx_d[:])
            slots_all = cp.tile([P, n_groups], bf16)
            nc.sync.dma_start(slots_all[:], slot_d[:])
            if ts_oh:
                slots_f = cp.tile([P, n_groups], f32)
                nc.scalar.copy(slots_f[:], slots_all[:])

            def sub_ag(shard_t, table_t, r, tag):
                nc.gpsimd.collective_compute(
                    "AllGather", mybir.AluOpType.bypass, replica_groups=rg,
                    ins=[shard_t[rstart[r]:rstart[r] + rs[r], :].opt()],
                    outs=[table_t[chunk_bases[r]:
                                  chunk_bases[r] + chunk_rows[r], :].opt()])

            # ---- layer-1 transform: h1' = dinv * (x @ W1), node-major, bf16.
            xT_r = xT_d.rearrange("(c p) n -> p c n", p=P)
            pairs = [(t, min(t + 2, nb)) for t in range(0, nb, 2)]
            ag_done = 0
            for t0, t1 in pairs:
                w = (t1 - t0) * P
                xt = sb.tile([P, kc, 2 * P], bf16, tag="xt")
                nc.sync.dma_start(xt[:, :, :w],
                                  xT_r[:, :, t0 * P:t0 * P + w])
                hT = pp.tile([HID, 2 * P], f32, tag="ps")
                for c in range(kc):
                    nc.tensor.matmul(out=hT[:, :w], lhsT=w1[:, c, :],
                                     rhs=xt[:, c, :w],
                                     start=(c == 0), stop=(c == kc - 1))
                hTs = sb.tile([HID, 2 * P], f32, tag="hTs")
                nc.scalar.copy(hTs[:, :w], hT[:, :w])
                for t in range(t0, t1):
                    off = (t - t0) * P
                    h = pp.tile([P, HID], f32, tag="ps")
                    nc.tensor.transpose(out=h[:], in_=hTs[:, off:off + P],
                                        identity=ident[:HID, :HID])
                    hp = sb.tile([P, HID], bf16, tag="hp")
                    nc.scalar.mul(hp[:], h[:], mul=dinv_sb[:, t:t + 1])
                    nc.sync.dma_start(shard1[t * P:(t + 1) * P, :HID], hp[:])
                while ag_done < n_chunks and t1 >= rend_blk[ag_done]:
                    if stage != 0:
                        sub_ag(shard1, table1, ag_done, "ag1")
                    ag_done += 1

            if stage == 0:
                for b in range(nb):
                    t0_ = sb.tile([P, HID], bf16, tag="cp")
                    nc.sync.dma_start(t0_[:], shard1[b * P:(b + 1) * P, :HID])
                    o_ = sb.tile([P, HID], f32, tag="cpo")
                    nc.scalar.copy(o_[:], t0_[:])
                    nc.sync.dma_start(out_d[b * P:(b + 1) * P, :], o_[:])

            def aggregate(table, layer, gather_only=False, finish=None,
                          after_block=None):
                colpos = 0
                gpos = 0
                qsel = 0
                acc = ap_.tile([P, nb * HID], f32, tag=f"acc{layer}",
                               name=f"acc{layer}")
                done = []
                for c, blocks in calls:
                    qn = sum(g for _, _, g in blocks)
                    gidx_sb = gidx_all[:, colpos:colpos + 8 * qn]
                    colpos += 8 * qn
                    slots_sb = slots_all[:, gpos:gpos + qn]
                    gt = gp.tile([P, qn, TW], bf16, tag="g")
                    nc.gpsimd.dma_gather(
                        out_ap=gt[:],
                        in_ap=table[chunk_bases[c]:
                                    chunk_bases[c] + chunk_rows[c], :],
                        idxs_ap=gidx_sb, num_idxs=P * qn,
                        num_idxs_reg=P * qn, elem_size=TW,
                        single_packet=False, queue_num=qsel)
                    qsel = (qsel + 1) % nq
                    if gather_only:
                        gpos += qn
                        continue
                    for b, lo, g in blocks:
                        ps = pp.tile([P, HID], f32, tag="ps")
                        oh = ohp.tile([P, g, P], bf16, tag="oh")
                        if ts_oh:
                            for q in range(g):
                                nc.vector.tensor_scalar(
                                    out=oh[:, q, :], in0=iota_sb[:],
                                    scalar1=slots_f[:, gpos + lo + q:
                                                    gpos + lo + q + 1],
                                    scalar2=None,
                                    op0=mybir.AluOpType.is_equal)
                        else:
                            nc.vector.tensor_tensor(
                                out=oh[:],
                                in0=iota_sb[:].rearrange("p (g j) -> p g j", g=1)
                                    .to_broadcast([P, g, P]),
                                in1=slots_sb[:, lo:lo + g]
                                    .rearrange("p (g j) -> p g j", j=1)
                                    .to_broadcast([P, g, P]),
                                op=mybir.AluOpType.is_equal)
                        for q in range(lo, lo + g):
                            nc.tensor.matmul(out=ps[:], lhsT=oh[:, q - lo, :],
                                             rhs=gt[:, q, :HID],
                                             start=(q == lo),
                                             stop=(q == lo + g - 1))
                        a_sl = acc[:, b * HID:(b + 1) * HID]
                        if c == first_c[b]:
                            nc.scalar.copy(a_sl, ps[:])
                        else:
                            nc.vector.tensor_add(out=a_sl, in0=a_sl,
                                                 in1=ps[:])
                        if c == last_c[b]:
                            if finish is not None:
                                finish(b, a_sl)
                            if after_block is not None:
                                after_block(b)
                            if finish is None:
                                done.append((b, a_sl))
                    gpos += qn
                return done

            # ---- layer-1 aggregation + layer-2 transform (inline finish)
            def finish1(b, a_sl):
                accb = rp.tile([P, HID], f32, tag="accb", name="accb")
                nc.vector.tensor_scalar(
                    out=accb[:], in0=a_sl, scalar1=dinv_sb[:, b:b + 1],
                    scalar2=None, op0=mybir.AluOpType.mult)
                acc2 = rp.tile([P, HID], f32, tag="acc2", name="acc2")
                nc.vector.tensor_add(out=acc2[:], in0=accb[:], in1=b1_bc[:])
                h2 = sb.tile([P, HID], f32, tag="h2", bufs=6, name="h2")
                nc.scalar.activation(h2[:], acc2[:],
                                     mybir.ActivationFunctionType.Relu)
                h2T = pp.tile([HID, P], f32, tag="ps", name="h2T")
                nc.tensor.transpose(out=h2T[:], in_=h2[:], identity=ident[:])
                h2Ts = sb.tile([HID, P], f32, tag="h2Ts", bufs=6, name="h2Ts")
                nc.scalar.copy(h2Ts[:], h2T[:])
                gT = pp.tile([HID, P], f32, tag="ps", name="gT")
                nc.tensor.matmul(out=gT[:], lhsT=w2[:], rhs=h2Ts[:],
                                 start=True, stop=True)
                gTs = sb.tile([HID, P], f32, tag="gTs", bufs=6, name="gTs")
                nc.scalar.copy(gTs[:], gT[:])
                gg = pp.tile([P, HID], f32, tag="ps", name="gg")
                nc.tensor.transpose(out=gg[:], in_=gTs[:],
                                    identity=ident[:HID, :HID])
                gsb = sb.tile([P, HID], bf16, tag="gsb", bufs=6, name="gsb")
                nc.scalar.mul(gsb[:], gg[:], mul=dinv_sb[:, b:b + 1])
                nc.sync.dma_start(shard2[b * P:(b + 1) * P, :HID], gsb[:])

            def finish3(b, a_sl):
                ob = rp.tile([P, HID], f32, tag="ob", name="ob")
                nc.vector.tensor_scalar(
                    out=ob[:], in0=a_sl, scalar1=dinv_sb[:, b:b + 1],
                    scalar2=None, op0=mybir.AluOpType.mult)
                nc.sync.dma_start(out_d[b * P:(b + 1) * P, :], ob[:])

            ag2_state = {"done": 0}

            def after_block1(b):
                while (ag2_state["done"] < n_chunks
                       and b + 1 >= rend_blk[ag2_state["done"]]):
                    sub_ag(shard2, table2, ag2_state["done"], "ag2")
                    ag2_state["done"] += 1

            if stage == 2:
                aggregate(table1, 1, gather_only=True)
            elif stage == 3:
                aggregate(table1, 1, finish=finish3)
            elif stage >= 4:
                aggregate(table1, 1, finish=finish1, after_block=after_block1)

            # ---- layer-2 aggregation (inline final epilogue)
            def finish2(b, a_sl):
                acc4 = rp.tile([P, HID], f32, tag="acc4", name="acc4")
                nc.vector.tensor_scalar(
                    out=acc4[:], in0=a_sl, scalar1=dinv_sb[:, b:b + 1],
                    scalar2=None, op0=mybir.AluOpType.mult)
                osb = sb.tile([P, HID], f32, tag="osb", name="osb")
                nc.vector.tensor_add(out=osb[:], in0=acc4[:], in1=b2_bc[:])
                nc.sync.dma_start(out_d[b * P:(b + 1) * P, :], osb[:])

            if stage >= 4:
                aggregate(table2, 2, finish=finish2)

    nc.compile()
    return nc


# ------------------------------------------------------------------- driver
_CACHE = {}


def _get_nc(in_ch, meta):
    key = (in_ch, meta["shard"], meta["wtot"],
           tuple((c, tuple(b)) for c, bl in meta["calls"] for b in bl))
    if key not in _CACHE:
        _CACHE[key] = _build(in_ch, meta)
    return _CACHE[key]


def _in_maps(pre, W1, b1, W2, b2):
    import ml_dtypes
    maps = []
    for k in range(N_CORES):
        maps.append({
            "xT": pre["xts"][k],
            "gidx": pre["gidx"][k],
            "slots": pre["slots"][k],
            "iota": pre["iota"],
            "dinv": pre["dinv_cols"][k],
            "W1": np.ascontiguousarray(W1.astype(ml_dtypes.bfloat16)),
            "b1": b1, "W2": W2, "b2": b2,
        })
    return maps


def kernel(x, edge_index, W1, b1, W2, b2):
    from concourse.bass_utils import run_bass_kernel_spmd

    x = np.asarray(x, dtype=np.float32)
    W1 = np.ascontiguousarray(np.asarray(W1, dtype=np.float32))
    W2 = np.ascontiguousarray(np.asarray(W2, dtype=np.float32))
    b1 = np.asarray(b1, dtype=np.float32).reshape(1, HID)
    b2 = np.asarray(b2, dtype=np.float32).reshape(1, HID)

    pre = _preprocess(x, edge_index)
    nc = _get_nc(x.shape[1], pre)
    res = run_bass_kernel_spmd(nc, _in_maps(pre, W1, b1, W2, b2),
                               core_ids=list(range(N_CORES)))

    npc = pre["npc"]
    out = np.empty((pre["n_nodes"], HID), dtype=np.float32)
    for k in range(N_CORES):
        out[pre["sorted_nodes"][k]] = res.results[k]["out"][:npc]
    return out
